# revision 38
# baseline (speedup 1.0000x reference)
"""CRF negative-log-likelihood kernel for Trainium2 (8 NeuronCores).

Math: reference computes  partition - gold  where
  partition = sum_b log 1^T [prod_{t=511..1} (D_t A^T)] alpha_0,
  A = exp(T), D_t = diag(exp(e_t)), alpha_0 = exp(e_0);
  gold = sum emissions[b,s,tags]*m + sum T[tags[s],tags[s+1]]*m[:,1:].

Key idea (rank-1 segmentation): products of strictly positive matrices
contract to rank-1 at machine precision within ~10 steps.  Split the
511-step chain into NSEG=16 segments of L=32.  Each middle segment j is
M_j ~= a_j b_j^T / c_j with a_j = M_j 1 (forward vector chain),
b_j^T = 1^T M_j (backward chain), c_j = colsum(b_j); segments 0 and 15
contribute their single exact chain.  Per batch row
  log Z = sum_{p=0..14} log(b_{p+1}^T a_p) - sum_{j=1..14} log colsum(b_j)
          + L*(NSEG)*C0  (constant, see below).
All 30 chains are independent -> the scan is 32 wide steps of
[C,480] matmuls + elementwise multiplies instead of 256 narrow
latency-bound steps (validated: f64 segmentation error ~1e-13; a full
bf16 numpy model of this kernel matches the reference at rel 3.2e-6).

Numerical range: instead of data-driven renormalization, the host
pre-shifts the raw emissions by the constant -C0 ~ the mean per-step
log growth (and by -log(A^T 1) on the t=0 block so chains can all start
from ones), so E = exp(shifted) keeps every state in a bounded random
walk (measured |log| < 40 vs bf16 range 88).  Compensation is the exact
constant C0 per consumed slice plus an integer-count term on the emit
gold; segment-internal factors cancel identically.  (A device-side
bias AP on the activation raced on cold hardware; host folding is the
safe equivalent.)

Device mapping (data-parallel over batch, 32 rows per core):
  * fwd state AF [C, 15*32]: block j = chain of segment j (j=0..14);
    bwd state WB holds E-premultiplied states for j=1..15.  Per step:
    2 matmuls (PE, bf16, full 128 contraction) into PSUM, 2 elementwise
    multiplies with the step's emission slice (DVE; GPSIMD cannot read
    PSUM - neuronxcc rejects it).  DVE is the saturated engine.
  * Emissions arrive raw bf16 in an l-pair-major host layout
    (col = POS[t%L]*512 + (t//L)*32 + b, POS = 0,31,1,30,...) so that
    every per-step slice for BOTH directions is contiguous in ONE
    shared array: fwd step s reads cols [POS[s]*512, +480), bwd reads
    [POS[31-s]*512+32, +480).  exp runs on ScalarE in storage order,
    which is exactly first-use order; DMA likewise.
  * gold emit: 128 accumulating PE matmuls hemit_chunk^T @ raw_chunk
    into one PSUM bank; sum(diag) = the masked gather-sum, extracted
    with an identity multiply + colsum.  gold trans: count matrix . T.
Host does integer/layout preprocessing and f64 log-sum postprocessing.
"""

import sys

for _p in ("/opt/trn_rl_repo",):
    if _p not in sys.path:
        sys.path.insert(0, _p)

import numpy as np
import ml_dtypes
from contextlib import ExitStack

from concourse import bass, tile, mybir, bacc
from concourse.bass_utils import run_bass_kernel_spmd
from concourse.tile_rust import add_dep_helper

NCORES = 8
B, S, C = 256, 512, 128
BC = B // NCORES          # 32 batch rows per core
NSEG = 16
L = S // NSEG             # 32 scan steps
NBLK = NSEG - 1           # 15 chains per direction
WF = NBLK * BC            # 480 state columns per direction
BLKW = NSEG * BC          # 512: one l-position across all segments
EFREE = S * BC            # 16384 emission columns (single copy)
C0 = 5.375                # constant per-step log-growth compensation

# storage position of l-value: l-pairs (0,31),(1,30),... in need order
POS = [0] * L
for _l in range(L):
    POS[_l] = 2 * _l if _l < L // 2 else 2 * (L - 1 - _l) + 1

# DMA/exp chunk boundaries in storage *positions* (each position = 512 cols);
# position m is first needed at scan step m//2.
CH_POS = [0, 1, 2, 3, 4] + list(range(6, 34, 2))
NCH = len(CH_POS) - 1
EXP_LEAD = 6              # inject exp of chunk k this many steps early

F32 = mybir.dt.float32
BF16 = mybir.dt.bfloat16
AF = mybir.ActivationFunctionType
OP = mybir.AluOpType

_NC_CACHE = None


def _fwd_off(s):
    return POS[s] * BLKW


def _bwd_off(s):
    # bwd mult at loop iter s consumes l = 31-s; block j=0 excluded
    return POS[L - 1 - s] * BLKW + BC


def _build_nc():
    nc = bacc.Bacc("TRN2", target_bir_lowering=False, debug=False)

    eraw_in = nc.dram_tensor("eraw", [C, EFREE], BF16, kind="ExternalInput").ap()
    afwd = nc.dram_tensor("afwd", [C, C], BF16, kind="ExternalInput").ap()
    abwd = nc.dram_tensor("abwd", [C, C], BF16, kind="ExternalInput").ap()
    hem_in = nc.dram_tensor("hem", [C, EFREE], BF16, kind="ExternalInput").ap()
    cnt_in = nc.dram_tensor("cnt", [C, C], F32, kind="ExternalInput").ap()
    tsb_in = nc.dram_tensor("tsb", [C, C], F32, kind="ExternalInput").ap()
    id_in = nc.dram_tensor("ident", [C, C], BF16, kind="ExternalInput").ap()

    # single combined output row: [dots | bcol | emit | trans]
    OUTW = 2 * WF + 2 * C
    outs_o = nc.dram_tensor("outs", [1, OUTW], F32, kind="ExternalOutput").ap()

    with tile.TileContext(nc) as tc, ExitStack() as ctx:
        sb = ctx.enter_context(tc.tile_pool(name="sb", bufs=1))
        wk = ctx.enter_context(tc.tile_pool(name="wk", bufs=4))
        ps = ctx.enter_context(tc.tile_pool(name="ps", bufs=2, space="PSUM"))

        # ---- first emission chunks, then weights, then the stream -------
        wf = sb.tile([C, C], BF16, name="wf")
        wb_ = sb.tile([C, C], BF16, name="wb")
        eraw_sb = sb.tile([C, EFREE], BF16, name="eraw_sb")
        em_dmas = []

        def em_dma(k):
            o, e = CH_POS[k] * BLKW, CH_POS[k + 1] * BLKW
            em_dmas.append(nc.sync.dma_start(eraw_sb[:, o:e], eraw_in[:, o:e]))

        em_dma(0)
        em_dma(1)
        nc.sync.dma_start(wf[:], afwd[:])
        nc.sync.dma_start(wb_[:], abwd[:])
        for k in range(2, NCH):
            em_dma(k)

        ones_col = sb.tile([C, 1], BF16, name="ones_col")
        ones_row = sb.tile([1, C], BF16, name="ones_row")
        nc.vector.memset(ones_col[:], 1.0)
        nc.vector.memset(ones_row[:], 1.0)

        # gold + tail-only inputs stream after the emission chunks
        hem_sb = sb.tile([C, EFREE], BF16, name="hem_sb")
        anchor = em_dmas[-1].ins
        qs = EFREE // 8
        for k in range(8):
            gd = nc.sync.dma_start(hem_sb[:, k * qs:(k + 1) * qs],
                                   hem_in[:, k * qs:(k + 1) * qs])
            add_dep_helper(gd.ins, anchor, reason="gold DMA after emissions")
        cnt_sb = sb.tile([C, C], F32, name="cnt_sb")
        tsb_sb = sb.tile([C, C], F32, name="tsb_sb")
        id_sb = sb.tile([C, C], BF16, name="id_sb")
        for gd in (nc.sync.dma_start(cnt_sb[:], cnt_in[:]),
                   nc.sync.dma_start(tsb_sb[:], tsb_in[:]),
                   nc.sync.dma_start(id_sb[:], id_in[:])):
            add_dep_helper(gd.ins, anchor, reason="tail DMA after emissions")

        # ---- exp chunks on ScalarE.  The -C0 range bias and the chain-0
        # seed correction (-log A^T 1 on cols 0:BC) are folded into the
        # host-prepared raw data, so this is a plain exp.
        E = sb.tile([C, EFREE], BF16, name="E")
        warm = sb.tile([C, 1], BF16, name="warm")
        nc.scalar.activation(warm[:], ones_col[:], AF.Exp)

        def exp_chunk(k):
            o, e = CH_POS[k] * BLKW, CH_POS[k + 1] * BLKW
            nc.scalar.activation(E[:, o:e], eraw_sb[:, o:e], AF.Exp)

        exp_chunk(0)

        exp_chunk(1)

        # ---- scan -------------------------------------------------------
        af = wk.tile([C, WF], BF16, tag="af", bufs=3, name="af_init")
        nc.vector.memset(af[:], 1.0)

        emit_ps = ps.tile([C, C], F32, tag="emit", bufs=1, name="emit_ps")
        NEMIT = EFREE // C
        emit_n = [0]

        def emit_mm():
            g = emit_n[0]
            if g >= NEMIT:
                return
            emit_n[0] += 1
            nc.tensor.matmul(emit_ps[:], hem_sb[:, g * C:(g + 1) * C],
                             eraw_sb[:, g * C:(g + 1) * C],
                             start=(g == 0), stop=(g == NEMIT - 1))

        outs_sb = sb.tile([1, OUTW], F32, name="outs_sb")
        wbst = None  # bwd SBUF state; step 0 feeds the E slice directly
        for s in range(L):
            ppF = ps.tile([C, WF], F32, tag="ppF", bufs=2, name=f"ppF{s}")
            nc.tensor.matmul(ppF[:], wf[:], af[:], start=True, stop=True)
            brhs = wbst[:] if wbst is not None \
                else E[:, _bwd_off(0):_bwd_off(0) + WF]
            ppB = ps.tile([C, WF], F32, tag="ppB", bufs=2, name=f"ppB{s}")
            nc.tensor.matmul(ppB[:], wb_[:], brhs, start=True, stop=True)

            o = _fwd_off(s)
            af_new = wk.tile([C, WF], BF16, tag="af", bufs=3, name=f"af{s + 1}")
            nc.vector.tensor_tensor(af_new[:], ppF[:], E[:, o:o + WF],
                                    op=OP.mult)
            af = af_new

            if s < L - 1:
                o = _bwd_off(s + 1)
                wb_new = wk.tile([C, WF], BF16, tag="wbs", bufs=3,
                                 name=f"wbs{s + 1}")
                nc.vector.tensor_tensor(wb_new[:], ppB[:], E[:, o:o + WF],
                                        op=OP.mult)
                wbst = wb_new

            # inject exp chunks ahead of need (chunk k live at step
            # CH_POS[k]//2)
            for k in range(2, NCH):
                if s == max(0, CH_POS[k] // 2 - EXP_LEAD):
                    exp_chunk(k)
            # emit-gold matmuls fill PE idle slots once hemit streams in
            if s >= 4:
                for _ in range(6):
                    emit_mm()

            if s == 28:
                # trans gold (independent of the scan): colsum(cnt * T)
                tt = wk.tile([C, C], F32, tag="tt", bufs=1, name="tt")
                nc.gpsimd.tensor_tensor(tt[:], cnt_sb[:], tsb_sb[:],
                                        op=OP.mult)
                ttb = wk.tile([C, C], BF16, tag="ide", bufs=2, name="ttb")
                nc.gpsimd.tensor_copy(ttb[:], tt[:])
                tps = ps.tile([1, C], F32, tag="cs", bufs=2, name="trans_cs")
                nc.tensor.matmul(tps[:], ones_col[:], ttb[:],
                                 start=True, stop=True)
                nc.scalar.copy(outs_sb[0:1, 2 * WF + C:], tps[:])
            if s == 29:
                # emit gold: sum(diag(emit_ps)) via identity mask + colsum
                assert emit_n[0] >= NEMIT
                ide = wk.tile([C, C], BF16, tag="ide", bufs=2, name="ide")
                nc.vector.tensor_tensor(ide[:], emit_ps[:], id_sb[:],
                                        op=OP.mult)
                eps2 = ps.tile([1, C], F32, tag="cs", bufs=2, name="emit_cs")
                nc.tensor.matmul(eps2[:], ones_col[:], ide[:],
                                 start=True, stop=True)
                nc.scalar.copy(outs_sb[0:1, 2 * WF:2 * WF + C], eps2[:])
        # ---- tail: boundary dots off the final states -------------------
        # block p of both states holds the segment pair (p+1, p)
        d = wk.tile([C, WF], BF16, tag="es", bufs=2, name="dmeet")
        nc.vector.tensor_tensor(d[:], ppB[:], af[:], op=OP.mult)
        bfin = wk.tile([C, WF], BF16, tag="bfin", bufs=1, name="bfin")
        nc.scalar.copy(bfin[:], ppB[:])
        dps = ps.tile([1, WF], F32, tag="cs", bufs=2, name="dots_ps")
        nc.tensor.matmul(dps[:], ones_col[:], d[:], start=True, stop=True)
        nc.scalar.copy(outs_sb[0:1, 0:WF], dps[:])
        csB = ps.tile([1, WF], F32, tag="cs", bufs=2, name="csB_fin")
        nc.tensor.matmul(csB[:], ones_col[:], bfin[:], start=True, stop=True)
        nc.vector.tensor_copy(outs_sb[0:1, WF:2 * WF], csB[:])

        nc.sync.dma_start(outs_o[:], outs_sb[:])

    nc.compile()
    return nc


def _prep_inputs(emissions, tags, mask, transitions):
    em = np.asarray(emissions, dtype=np.float32)
    tg = np.asarray(tags).astype(np.int64)
    mk = np.asarray(mask).astype(np.float32)
    tr = np.ascontiguousarray(np.asarray(transitions, dtype=np.float32))

    a_f = np.exp(tr.astype(np.float64))
    afwd = a_f.astype(ml_dtypes.bfloat16)
    abwd = np.ascontiguousarray(a_f.T).astype(ml_dtypes.bfloat16)
    ident = np.eye(C, dtype=ml_dtypes.bfloat16)
    # chain-0 seed: divide E_0 by A^T 1 (in log space) so the uniform
    # ones start reproduces alpha_0 exactly; -C0 range bias on everything
    aw_host = afwd.astype(np.float64).sum(axis=0)
    seed_shift = np.log(aw_host).astype(np.float32).reshape(C, 1)

    # t_idx[m, j] = j*L + O[m]: storage position m holds l-value O[m]
    O = np.empty(L, dtype=np.int64)
    for l in range(L):
        O[POS[l]] = l
    t_idx = np.arange(NSEG)[None, :] * L + O[:, None]    # [m, j]

    in_maps = []
    emit_comp = []
    for core in range(NCORES):
        b0 = core * BC
        ecsb = np.ascontiguousarray(em[b0:b0 + BC].transpose(2, 1, 0))
        esh = ecsb - C0                                  # [C, S, BC]
        esh[:, 0, :] -= seed_shift
        eraw = np.ascontiguousarray(
            esh[:, t_idx, :].reshape(C, EFREE)).astype(ml_dtypes.bfloat16)

        tgc = tg[b0:b0 + BC]                             # [BC, S]
        mkc = mk[b0:b0 + BC]
        hfull = np.zeros((C, S, BC), dtype=np.float32)
        s_all = np.arange(S)
        b_all = np.arange(BC)
        bb, ss = np.meshgrid(b_all, s_all, indexing="ij")
        hfull[tgc.ravel(), ss.ravel(), bb.ravel()] = mkc.ravel()
        hem = np.ascontiguousarray(
            hfull[:, t_idx, :].reshape(C, EFREE)).astype(ml_dtypes.bfloat16)

        cntm = np.zeros((C, C), dtype=np.float64)
        np.add.at(cntm, (tgc[:, :-1].ravel(), tgc[:, 1:].ravel()),
                  mkc[:, 1:].ravel().astype(np.float64))

        in_maps.append({
            "eraw": eraw, "afwd": afwd, "abwd": abwd, "hem": hem,
            "cnt": cntm.astype(np.float32), "tsb": tr, "ident": ident,
        })
        # emit-gold compensation for the host-folded shifts: every masked
        # (t,b) lost C0, and (t=0,b) additionally lost seed_shift[tag]
        comp = C0 * float(mkc.sum())
        comp += float((mkc[:, 0] * seed_shift.ravel()[tgc[:, 0]]).sum())
        emit_comp.append(comp)
    return in_maps, emit_comp


def kernel(emissions, tags, mask, transitions, _trace=False):
    global _NC_CACHE
    if _NC_CACHE is None:
        _NC_CACHE = _build_nc()
    nc = _NC_CACHE

    in_maps, emit_comp = _prep_inputs(emissions, tags, mask, transitions)

    def run_once():
        res = run_bass_kernel_spmd(
            nc, in_maps, core_ids=list(range(NCORES)), trace=_trace,
        )
        partition = np.float64(0.0)
        gold = np.float64(0.0)
        ok = True
        for core, r in enumerate(res.results):
            outs = np.asarray(r["outs"], dtype=np.float64).ravel()
            dots = outs[0:WF].reshape(NBLK, BC)
            bcol = outs[WF:2 * WF].reshape(NBLK, BC)
            if not (np.all(dots > 0) and np.all(bcol[:NBLK - 1] > 0)
                    and np.all(np.isfinite(outs))):
                ok = False
                break
            partition += np.log(dots).sum()
            partition -= np.log(bcol[:NBLK - 1]).sum()
            partition += C0 * L * (NBLK + 1) * BC
            gold += outs[2 * WF:].sum() + emit_comp[core]
        return ok, np.float32(partition - gold), res

    ok, out, res = run_once()
    if not ok:
        # first-ever execution on a cold device can race input staging;
        # rerun (deterministic) and validate again
        ok, out, res = run_once()
    if _trace:
        return out, res
    return out
